# revision 23
# baseline (speedup 1.0000x reference)
"""CRF loss (forward-algorithm partition + gold energy) on 8 TRN2 NeuronCores.

Strategy (data-parallel over batch, per the sharding hint):
  - batch 64 -> 8 cores x 8 local batches.
  - Forward recurrence kept in the *linear* domain: state q[t', b] with
    partition[b, t'] = ln q[t', b] + sum_k ln(m_k[b]).  One step is
    q <- E_b^T q per local batch (E = exp(scores[s,b])), computed as 8 tiny
    PE matvecs against bf16 E tiles produced by one big ScalarE exp per
    chunk of timesteps.  exp/log of the textbook logsumexp cancel between
    steps, so ScalarE only exponentiates each score element once.
  - Every 8 steps the state is renormalized by its column sum (computed with
    a ones-vector matmul; scaling broadcast across partitions with a rank-1
    matmul), and the sum is stashed in SBUF.  The last renorm sits 13 steps
    before the end so its serial chain stays off the drain (growth is
    ~e^5.4/step and bf16 tops out at ~e^88, so 13 unrenormalized steps are
    safe).
  - The loss only reads q_final[END_TAG], so the LAST timestep needs just
    the to=END_TAG column of its scores: the tail update is an elementwise
    multiply + ones-matvec on a [128, 8] tile instead of a full 0.5 MB
    score chunk (shrinks the final DMA+exp+matvec drain).
  - The device never takes logs: the final column state and the stashed f32
    renorm sums are DMAed out and the logs happen on the host.  No Ln table
    load, no Ln on the serial tail.
  - Gold-path energy is a 16K-element gather -- pure host numpy (a device
    indirect-DMA version stalled the score stream mid-kernel).
  - Timeline: the score stream is the bottleneck (DMA-roofline), so the
    first score-chunk DMA is issued before anything else, chunks are 8
    timesteps in the bulk (amortizes ScalarE instruction overhead) and
    taper to 4/2 at the end; tail chunks draw from their own tile pool so
    their DMAs never wait on a bulk buffer.
"""

import numpy as np

import concourse.bacc as bacc
import concourse.bass as bass
import concourse.mybir as mybir
import concourse.tile as tile
from concourse import bass_utils

S = 256
B = 64
T = 128
NCORES = 8
BL = B // NCORES  # 8 local batches per core
START_TAG = 126
END_TAG = 127
CHUNK = 8  # bulk timesteps per score DMA + exp instruction
RENORM_EVERY = 8

f32 = mybir.dt.float32
bf16 = mybir.dt.bfloat16
u8 = mybir.dt.uint8
Exp = mybir.ActivationFunctionType.Exp
Alu = mybir.AluOpType


def renorm_steps(n_steps):
    # last renorm >= 13 steps before the end: off the serial drain, within
    # bf16 range (see module docstring).
    return [s for s in range(2, n_steps - 13, RENORM_EVERY)]


# tail chunk sizes (last DMAs): small so the serial matvec chain catches up
# to the stream before it ends; overridable via K_TAIL for schedule sweeps
TAIL = [5, 2, 2, 2, 1, 1]


def chunk_schedule(n_steps):
    """(start_step, n_sub) covering full-matrix steps 1..n_steps-2."""
    import os

    total = n_steps - 2
    sizes = []
    tail = TAIL
    if os.environ.get("K_TAIL"):
        tail = [int(x) for x in os.environ["K_TAIL"].split(",")]
    tail = [n for n in tail if n > 0]
    sizes = []
    while total > sum(tail):
        n = CHUNK if total - CHUNK >= sum(tail) else total - sum(tail)
        sizes.append(n)
        total -= n
    sizes += list(tail)
    assert sum(sizes) == n_steps - 2
    out = []
    s = 1
    for n in sizes:
        out.append((s, n))
        s += n
    assert s == n_steps - 1
    return out


def build(n_steps=S):
    """Build + compile the SPMD kernel for one core's batch shard."""
    nrn = renorm_steps(n_steps)
    nc = bacc.Bacc(
        "TRN2", target_bir_lowering=False, debug=False, num_devices=NCORES
    )
    sc = nc.dram_tensor("scores", [n_steps - 2, T, BL, T], f32, kind="ExternalInput")
    p0 = nc.dram_tensor("p0t", [T, BL], f32, kind="ExternalInput").ap()
    lc = nc.dram_tensor("lastcol", [T, BL], f32, kind="ExternalInput").ap()
    eh = nc.dram_tensor("endrow", [1, T], bf16, kind="ExternalInput").ap()
    mk = nc.dram_tensor(
        "masks", [T, (n_steps - 1) * BL], u8, kind="ExternalInput"
    ).ap()
    o_q = nc.dram_tensor("out_q", [1, BL], f32, kind="ExternalOutput").ap()
    o_m = None
    if nrn:
        o_m = nc.dram_tensor(
            "out_msum", [1, len(nrn) * BL], f32, kind="ExternalOutput"
        ).ap()

    with tile.TileContext(nc) as tc:
        _body(nc, tc, sc, p0, lc, eh, mk, o_q, o_m, n_steps, nrn)
    nc.compile()
    return nc


def _body(nc, tc, sc, p0, lc, eh, mk, o_q, o_m, n_steps, nrn):
    import os
    from contextlib import ExitStack

    nomasks = os.environ.get("K_NOMASKS")
    norenorm = os.environ.get("K_NORENORM")
    noexp = os.environ.get("K_NOEXP")
    nomm = os.environ.get("K_NOMM")
    repeat = int(os.environ.get("K_REPEAT", "1"))

    sc_ap = sc.ap()
    chunks = chunk_schedule(n_steps)
    last_step = n_steps - 1

    with ExitStack() as ctx:
        const = ctx.enter_context(tc.tile_pool(name="const", bufs=1))
        spool = ctx.enter_context(tc.tile_pool(name="spool", bufs=3))
        tpool = ctx.enter_context(tc.tile_pool(name="tpool", bufs=4))
        epool = ctx.enter_context(tc.tile_pool(name="epool", bufs=3))
        vpool = ctx.enter_context(tc.tile_pool(name="vpool", bufs=4, space="PSUM"))
        rpool = ctx.enter_context(tc.tile_pool(name="rpool", bufs=2, space="PSUM"))
        small = ctx.enter_context(tc.tile_pool(name="small", bufs=2))

        # ---- the score stream is the roofline: start it immediately ----
        s0, n0 = chunks[0]
        first_sc = spool.tile([128, n0 * BL * T], f32, tag="sc")
        nc.sync.dma_start(
            out=first_sc[:],
            in_=sc_ap[s0 - 1 : s0 - 1 + n0].rearrange("s t b u -> t s b u"),
        )

        # ---- constants & persistent state ----
        p0_sb = small.tile([128, BL], f32)
        nc.sync.dma_start(out=p0_sb[:], in_=p0[:])
        lc_sb = small.tile([128, BL], f32)
        nc.sync.dma_start(out=lc_sb[:], in_=lc[:])
        eh_row = const.tile([1, 128], bf16)  # one-hot row at END_TAG
        nc.sync.dma_start(out=eh_row[:], in_=eh[:])
        masks_sb = const.tile([128, (n_steps - 1) * BL], u8)
        nc.sync.dma_start(out=masks_sb[:], in_=mk[:])
        ones_col = const.tile([128, 1], bf16)
        nc.vector.memset(ones_col[:], 1.0)
        ones_row = const.tile([1, 128], f32)
        nc.vector.memset(ones_row[:], 1.0)
        q = const.tile([128, BL], bf16)  # recurrence state
        ecol = const.tile([128, BL], bf16)  # exp of last-step END column
        ones8 = const.tile([1, BL], bf16)
        nc.vector.memset(ones8[:], 1.0)
        col = const.tile([128, BL], bf16)  # final-step column selector
        mbuf = None
        if nrn and not nomm:
            mbuf = const.tile([1, len(nrn) * BL], f32)  # stashed renorm sums

        # ---- main recurrence over timesteps 1..n_steps-2 ----
        nrn_set = set(nrn)
        for rep in range(repeat):
            nc.scalar.activation(out=q[:], in_=p0_sb[:], func=Exp)
            nc.scalar.activation(out=ecol[:], in_=lc_sb[:], func=Exp)
            # col[:, b] = mask[last, b] ? exp(lastcol[:, b]) : onehot(END_TAG)
            # (precomputed off the critical path; makes the final-step update
            # a single multiply + ones-matvec with no tail predication)
            if not nomm:
                col_ps = rpool.tile([128, BL], f32, tag="rbc")
                nc.tensor.matmul(
                    out=col_ps[:], lhsT=eh_row[:], rhs=ones8[:],
                    start=True, stop=True,
                )
                nc.vector.tensor_copy(out=col[:], in_=col_ps[:])
                if not nomasks:
                    nc.vector.copy_predicated(
                        out=col[:],
                        mask=masks_sb[:, (last_step - 1) * BL : last_step * BL],
                        data=ecol[:],
                    )
                else:
                    nc.vector.tensor_copy(out=col[:], in_=ecol[:])
            k_renorm = 0
            for ci, (s, nsub) in enumerate(chunks):
                if rep == 0 and ci == 0:
                    sc_tile = first_sc
                else:
                    pool = tpool if nsub <= 2 else spool
                    sc_tile = pool.tile(
                        [128, nsub * BL * T], f32,
                        tag="sct" if nsub <= 2 else "sc",
                    )
                    nc.sync.dma_start(
                        out=sc_tile[:],
                        in_=sc_ap[s - 1 : s - 1 + nsub].rearrange(
                            "s t b u -> t s b u"
                        ),
                    )
                if noexp:
                    e_tile = sc_tile.bitcast(bf16)[:, : nsub * BL * T]
                else:
                    e_tile = epool.tile([128, nsub * BL * T], bf16, tag="e")
                    nc.scalar.activation(out=e_tile[:], in_=sc_tile[:], func=Exp)
                for sl in range(nsub):
                    step = s + sl
                    if nomm:
                        continue
                    v = vpool.tile([128, BL], f32, tag="v")
                    for b in range(BL):
                        off = (sl * BL + b) * T
                        nc.tensor.matmul(
                            out=v[:, b : b + 1],
                            lhsT=e_tile[:, off : off + T],
                            rhs=q[:, b : b + 1],
                            start=True,
                            stop=True,
                        )
                    # q <- v where mask_for_padding[step] else q
                    if nomasks:
                        nc.vector.tensor_copy(out=q[:], in_=v[:])
                    else:
                        nc.vector.copy_predicated(
                            out=q[:],
                            mask=masks_sb[:, (step - 1) * BL : step * BL],
                            data=v[:],
                        )
                    if step in nrn_set and not norenorm:
                        ssum = rpool.tile([1, BL], f32, tag="sum")
                        nc.tensor.matmul(
                            out=ssum[:],
                            lhsT=ones_col[:],
                            rhs=q[:],
                            start=True,
                            stop=True,
                        )
                        if mbuf is not None:
                            nc.vector.tensor_copy(
                                out=mbuf[:, k_renorm * BL : (k_renorm + 1) * BL],
                                in_=ssum[:],
                            )
                        r_row = small.tile([1, BL], f32, tag="rrow")
                        nc.vector.reciprocal(out=r_row[:], in_=ssum[:])
                        r_bc = rpool.tile([128, BL], f32, tag="rbc")
                        nc.tensor.matmul(
                            out=r_bc[:],
                            lhsT=ones_row[:],
                            rhs=r_row[:],
                            start=True,
                            stop=True,
                        )
                        nc.vector.tensor_tensor(
                            out=q[:], in0=q[:], in1=r_bc[:], op=Alu.mult
                        )
                        k_renorm += 1
                        # stream the stashed sums out as soon as complete;
                        # keeps the output DMA off the serial tail
                        if (
                            k_renorm == len(nrn)
                            and rep == repeat - 1
                            and o_m is not None
                            and mbuf is not None
                        ):
                            nc.sync.dma_start(out=o_m[:], in_=mbuf[:])

            # ---- final step: only the END_TAG column matters ----
            # qf[b] = sum_from col[from, b] * q[from, b]  (col bakes in the
            # last-step mask: exp column if masked-in, one-hot(END) if not)
            qf = small.tile([1, BL], f32, tag="qf")
            if nomm:
                nc.vector.memset(qf[:], 1.0)
            else:
                t_el = small.tile([128, BL], bf16, tag="tel")
                nc.vector.tensor_tensor(
                    out=t_el[:], in0=q[:], in1=col[:], op=Alu.mult
                )
                u = rpool.tile([1, BL], f32, tag="sum")
                nc.tensor.matmul(
                    out=u[:], lhsT=ones_col[:], rhs=t_el[:], start=True, stop=True
                )
                nc.vector.tensor_copy(out=qf[:], in_=u[:])
            if rep == repeat - 1:
                nc.sync.dma_start(out=o_q[:], in_=qf[:])
                if o_m is not None and mbuf is None:
                    z = small.tile([1, len(nrn) * BL], f32, tag="z")
                    nc.vector.memset(z[:], 1.0)
                    nc.sync.dma_start(out=o_m[:], in_=z[:])


def make_in_maps(scores, target, mask_gold, mask_pad, n_steps=S):
    """Host-side sharding/preprocessing -> per-core input dicts."""
    scores = np.asarray(scores, dtype=np.float32)
    mp = np.asarray(mask_pad).astype(np.uint8)
    in_maps = []
    for c in range(NCORES):
        b0 = c * BL
        sc_c = np.ascontiguousarray(
            scores[1 : n_steps - 1, b0 : b0 + BL].transpose(0, 2, 1, 3)
        )
        p0_c = np.ascontiguousarray(scores[0, b0 : b0 + BL, START_TAG, :].T)
        lc_c = np.ascontiguousarray(
            scores[n_steps - 1, b0 : b0 + BL, :, END_TAG].T
        )
        eh_c = np.zeros((1, T), dtype=np.float32)
        eh_c[0, END_TAG] = 1.0
        eh_c = eh_c.astype(mybir.dt.np(bf16))
        mrow = mp[1:n_steps, b0 : b0 + BL].reshape(-1)
        mk_c = np.ascontiguousarray(
            np.broadcast_to(mrow[None, :], (128, (n_steps - 1) * BL))
        )
        in_maps.append(
            {
                "scores": sc_c,
                "p0t": p0_c,
                "lastcol": lc_c,
                "endrow": eh_c,
                "masks": mk_c,
            }
        )
    return in_maps


def host_tg_energy(scores, target, mask_gold, n_steps=S):
    """Gold-path energy: flat gather + masked sum, pure numpy."""
    scores = np.asarray(scores, dtype=np.float32)
    target = np.asarray(target).astype(np.int64)
    mg = np.asarray(mask_gold).astype(np.float64)
    flat = scores[:n_steps].reshape(n_steps, B, T * T)
    tg = np.take_along_axis(flat, target[:n_steps], axis=2)[..., 0]
    return float((tg.astype(np.float64) * mg[:n_steps]).sum())


def combine(results, tg_energy, n_steps=S):
    """Host-side reduction of per-core partials -> scalar loss."""
    part = 0.0
    for r in results:
        part += float(np.log(r["out_q"][0].astype(np.float64)).sum())
        if "out_msum" in r:
            part += float(np.log(r["out_msum"].astype(np.float64)).sum())
    return np.float32((part - tg_energy) / B)


_NC_CACHE = {}


def kernel(scores, target, mask_for_gold, mask_for_padding):
    if "nc" not in _NC_CACHE:
        _NC_CACHE["nc"] = build(S)
    nc = _NC_CACHE["nc"]
    in_maps = make_in_maps(scores, target, mask_for_gold, mask_for_padding, S)
    res = bass_utils.run_bass_kernel_spmd(
        nc, in_maps, core_ids=list(range(NCORES))
    )
    tg = host_tg_energy(scores, target, mask_for_gold, S)
    return combine(res.results, tg, S)
